# revision 38
# baseline (speedup 1.0000x reference)
"""GAT attention layer (B=8, N=2048, C=512) on 8 TRN2 NeuronCores.

Data-parallel over B: core b handles graph b.
Per-core math (x: [N,C], w: [C,C], a: [2C,1]):
    wa_t = w @ a_t                      (t=0,1)       [C]
    s_t  = x @ wa_t                                   [N]
    p_ji = exp(leaky_relu(s1_i + s2_j))
    r_i  = sum_j p_ji;  out = (p^T @ x) / r

Softmax rows are invariant to any per-row scale, so with
lambda_i = exp(-0.2*s1_i) we compute
    p'_ji = (G_i * F1_j) max F1a_j
    G = exp(0.8*s1), F1 = exp(s2), F1a = exp(0.2*s2)
which is ONE DVE tensor_scalar op per score block.

v5: all partition-broadcasts (a rows, wa rows, G rows) run ON THE PE as
matmul-broadcasts: out[m,n] = sum_k sel[k,m]*src[k,n] with a host-fed
one-hot selector (or an all-ones row for 1-partition sources).  ~150ns
per 128-wide chunk into PSUM, then one ACT copy to SBUF.  This removes
the gpsimd custom-op library (~9us load), the row-gather DMA hops, and
the shared-DMA-semaphore waits that made those broadcasts take 4-8us
each.  i is processed in quarters (PSUM group g = k-chunks 4g..4g+3) so
scoring starts right after the first 4 s1 dots.

Engine roles:
  PE   : 256 PV MMs + 256 r-MMs + broadcast MMs (head, doubles as warmup)
  DVE  : wa/s dots (STT), 32x32 transposes, p' tensor_scalar, recips
  ACT  : x->fp16 casts, broadcast PSUM->SBUF copies, F/G exps, normalizes
  GPS  : plain x DMA ring only
"""

import sys

import numpy as np

if "/opt/trn_rl_repo" not in sys.path:
    sys.path.insert(0, "/opt/trn_rl_repo")

B, N, C = 8, 2048, 512
P = 128
NJ = N // P  # 16 source-node blocks
NQ = 512  # i-quarter width
ALPHA = 0.2  # leaky_relu slope
# PSUM: 8 banks = 4 output accumulators + 2 r (rotating) + 2 head scratch.
GROUPS = [(0, 4), (4, 8), (8, 12), (12, 16)]

_CACHE = {}


def _build():
    from contextlib import ExitStack

    import concourse.bacc as bacc
    import concourse.bass as bass
    import concourse.tile as tile
    from concourse import mybir

    fp32 = mybir.dt.float32
    f32r = mybir.dt.float32r
    fp16 = mybir.dt.float16
    bf16 = mybir.dt.bfloat16
    AF = mybir.ActivationFunctionType
    OP = mybir.AluOpType

    nc = bacc.Bacc("TRN2", target_bir_lowering=False)
    x_d = nc.dram_tensor("x", [N, C], fp32, kind="ExternalInput")
    w_d = nc.dram_tensor("w", [C, C], fp32, kind="ExternalInput")
    a_d = nc.dram_tensor("a", [2 * C, 1], fp32, kind="ExternalInput")
    sel_d = nc.dram_tensor("sel", [32, NJ * P], fp16, kind="ExternalInput")
    sel32_d = nc.dram_tensor("sel32", [32, NJ * P], fp32, kind="ExternalInput")
    o_d = nc.dram_tensor("o", [N, C], fp32, kind="ExternalOutput")

    with ExitStack() as ctx:
        tc = ctx.enter_context(tile.TileContext(nc))
        const = ctx.enter_context(tc.tile_pool(name="const", bufs=1))
        wpool = ctx.enter_context(tc.tile_pool(name="w", bufs=4))
        xpool = ctx.enter_context(tc.tile_pool(name="xin", bufs=NJ))
        xbfp = ctx.enter_context(tc.tile_pool(name="xbf", bufs=NJ))
        ppool = ctx.enter_context(tc.tile_pool(name="p", bufs=2 * NJ))
        scr = ctx.enter_context(tc.tile_pool(name="scr", bufs=6))
        osb = ctx.enter_context(tc.tile_pool(name="osb", bufs=3))
        ps_out = ctx.enter_context(tc.tile_pool(name="ps_out", bufs=4, space="PSUM"))
        ps_r = ctx.enter_context(tc.tile_pool(name="ps_r", bufs=2, space="PSUM"))
        ps_h = ctx.enter_context(tc.tile_pool(name="ps_h", bufs=2, space="PSUM"))

        # --- persistent small tiles -------------------------------------
        s1col = const.tile([P, 32], fp32)  # s1[128j+p] at [p, j]; cols 16+ pad
        s2col = const.tile([P, NJ], fp32)
        F1col = const.tile([P, NJ], fp32)  # exp(s2)
        F1acol = const.tile([P, NJ], fp32)  # exp(ALPHA*s2)
        GT = const.tile([32, P], fp32)  # raw s1 j 0..7 transposed
        GT2 = const.tile([32, P], fp32)  # j 8..15
        Gb = const.tile([P, N], fp16)  # G broadcast rows
        wa12 = const.tile([P, 32], fp32)  # wa_t[128q+p] at [p, t*4+q]
        waT1 = const.tile([32, P], fp16)  # rows 0:4 = wa1 chunks
        waT2 = const.tile([32, P], fp16)  # rows 4:8 = wa2 chunks
        wa16 = const.tile([P, 32], fp16)  # wa12 cast (transpose src)
        abc = const.tile([P, 2, C], fp16)  # a rows broadcast to 128 parts
        wab1 = const.tile([P, C], fp16)  # wa1 row broadcast to 128 parts
        wab2 = const.tile([P, C], fp16)  # wa2 row broadcast to 128 parts
        sel = const.tile([32, NJ * P], fp16)  # one-hot row selectors
        sel32 = const.tile([32, NJ * P], fp32)  # fp32 copy for fp32 bcasts
        ones_row = const.tile([1, P], fp16)  # K=1 lhsT for 1-row sources
        a_sb = const.tile([1, 2 * C], fp32)
        a16 = const.tile([1, 2 * C], fp16)
        ones_bf = const.tile([P, 1], bf16)
        warm_rhs = const.tile([P, C], bf16)
        rinv = const.tile([P, NJ], fp32)
        dummy = const.tile([P, 1], fp32)
        dummy2 = const.tile([P, 1], fp32)

        # --- t0: DMA ring kickoff ---------------------------------------
        a_row = a_d[:, 0].rearrange("(o k) -> o k", o=1)
        nc.scalar.dma_start(a_sb[:, 0:C], a_row[:, 0:C])
        nc.sync.dma_start(a_sb[:, C : 2 * C], a_row[:, C : 2 * C])
        wt = []
        w_engs = [nc.gpsimd, nc.gpsimd, nc.gpsimd, nc.gpsimd]
        for q in range(4):
            t = wpool.tile([P, C], fp32, tag="w")
            w_engs[q].dma_start(t[:], w_d[q * P : (q + 1) * P, :])
            wt.append(t)
        nc.sync.dma_start(sel[:], sel_d[:, :])
        nc.sync.dma_start(sel32[:], sel32_d[:, :])

        nc.vector.memset(ones_row[:], 1.0)
        # the broadcast MMs contract over all 32 partitions; unwritten
        # pad columns must be 0.0 (0*junk can be NaN)
        nc.vector.memset(s1col[:], 0.0)
        nc.vector.memset(wa16[:], 0.0)
        nc.vector.memset(ones_bf[:], 1.0)
        nc.vector.memset(warm_rhs[:], 0.0)

        # Preload ACT exp table during the DMA head phase.
        nc.scalar.activation(dummy[:], ones_bf[:], AF.Exp)

        # PE warm-up: back-to-back MMs trip the HAM activity window.
        warm_ps = ps_r.tile([P, C], fp32, tag="rps", name="warm_ps")
        for _ in range(6):
            nc.tensor.matmul(
                warm_ps[0:1, :],
                lhsT=ones_bf[:],
                rhs=warm_rhs[:],
                start=True,
                stop=True,
                skip_group_check=True,
            )

        def warm_on(rhs_ap):
            nc.tensor.matmul(
                warm_ps[0:1, :],
                lhsT=ones_bf[:],
                rhs=rhs_ap,
                start=True,
                stop=True,
                skip_group_check=True,
            )

        # --- PE matmul-broadcast helpers ---------------------------------
        def pe_bcast_row(dst, src_row, width):
            """Broadcast [1, width] fp16 src to [128, width] fp16 dst."""
            ps = ps_h.tile([P, width], fp32, tag="hb")
            nchunks = width // P
            for c in range(nchunks):
                nc.tensor.matmul(
                    ps[:, c * P : (c + 1) * P],
                    lhsT=ones_row[:],
                    rhs=src_row[:, c * P : (c + 1) * P],
                    start=(c == 0),
                    stop=(c == nchunks - 1),
                    skip_group_check=True,
                )
            nc.scalar.activation(dst, ps[:], AF.Copy)

        def pe_bcast_rows(dst, src32, rows, width, hi=False):
            """dst[128, width] fp16 <- concat of src32[r, :] for r in rows."""
            ps = ps_h.tile([P, width], fp32, tag="hb")
            rows = list(rows)
            for c, r in enumerate(rows):
                nc.tensor.matmul(
                    ps[:, c * P : (c + 1) * P],
                    lhsT=sel[:, r * P : (r + 1) * P],
                    rhs=src32[:],
                    start=(c == 0),
                    stop=(c == len(rows) - 1),
                    skip_group_check=True,
                )
            from contextlib import nullcontext
            with tc.high_priority() if hi else nullcontext():
                nc.scalar.activation(dst, ps[:], AF.Copy)

        # a -> abc: cast each half to fp16 as it lands, PE-broadcast
        nc.scalar.activation(a16[:, 0:C], a_sb[:, 0:C], AF.Copy)
        pe_bcast_row(abc[:, 0, :], a16[:, 0:C], C)
        nc.scalar.activation(a16[:, C : 2 * C], a_sb[:, C : 2 * C], AF.Copy)
        pe_bcast_row(abc[:, 1, :], a16[:, C : 2 * C], C)

        # --- x loads on three rings --------------------------------------
        xin = [xpool.tile([P, C], fp32, tag="xin", name=f"x_{j}") for j in range(NJ)]
        xbf = [xbfp.tile([P, C], fp16, tag="xbf", name=f"xb_{j}") for j in range(NJ)]
        def load_x(j):
            # all x on the gpsimd ring: the critical w/a/sel loads keep
            # their own per-ring DMA-completion semaphore batches, so the
            # wa/s dots stop waiting on unrelated bulk x transfers.
            nc.gpsimd.dma_start(xin[j][:], x_d[j * P : (j + 1) * P, :])

        def cast_x(j):
            nc.scalar.activation(xbf[j][:], xin[j][:], AF.Copy)
            if j % 4 == 0:
                warm_on(xbf[j][:])

        for j in range(NJ):
            load_x(j)
        for j in range(4):
            cast_x(j)

        # --- wa dots on DVE; col->row via 32x32 transposes + PE bcast ----
        def emit_wa_dots(t):
            for q in range(4):
                s = scr.tile([P, C], fp32, tag="ttr")
                nc.vector.scalar_tensor_tensor(
                    out=s[:],
                    in0=wt[q][:],
                    scalar=0.0,
                    in1=abc[:, t, :],
                    op0=OP.add,
                    op1=OP.mult,
                    accum_out=wa12[:, t * 4 + q : t * 4 + q + 1],
                )
            nc.vector.tensor_copy(wa16[:, t * 4 : t * 4 + 4], wa12[:, t * 4 : t * 4 + 4])
            waT = waT1 if t == 0 else waT2
            for b in range(4):
                nc.vector.transpose(waT[0:32, b * 32 : (b + 1) * 32],
                                    wa16[b * 32 : (b + 1) * 32, 0:32])
            wab = wab1 if t == 0 else wab2
            pe_bcast_rows(wab[:], waT, range(t * 4, t * 4 + 4), C)

        emit_wa_dots(0)
        emit_wa_dots(1)
        nc.vector.reciprocal(dummy2[:], dummy[:])  # preload DVE recip table

        # --- s dots (fp16 x * fp16 wab, fp32 accum) ----------------------
        def emit_s1(j):
            s = scr.tile([P, C], fp32, tag="ttr", name=f"s1scr_{j}")
            nc.vector.scalar_tensor_tensor(
                out=s[:],
                in0=xin[j][:],
                scalar=0.0,
                in1=wab1[:],
                op0=OP.add,
                op1=OP.mult,
                accum_out=s1col[:, j : j + 1],
            )

        def emit_s2(j, hi=False):
            s = scr.tile([P, C], fp32, tag="ttr", name=f"s2scr_{j}")
            nc.vector.scalar_tensor_tensor(
                out=s[:],
                in0=xin[j][:],
                scalar=0.0,
                in1=wab2[:],
                op0=OP.add,
                op1=OP.mult,
                accum_out=s2col[:, j : j + 1],
            )
            nc.scalar.activation(F1col[:, j : j + 1], s2col[:, j : j + 1], AF.Exp)
            nc.scalar.activation(
                F1acol[:, j : j + 1], s2col[:, j : j + 1], AF.Exp, scale=ALPHA
            )

        # G quarter chain: transpose RAW s1 cols (stays on DVE right after
        # the dots), PE-broadcast raw values, then exp(0.8*x) fused into the
        # PSUM->SBUF copy on ACT.  One fewer cross-engine hop than
        # exp-then-transpose.
        def emit_g_quarter(q):
            from contextlib import nullcontext
            gt = GT if q < 2 else GT2
            with tc.high_priority() if q < 2 else nullcontext():
                for b in range(4):
                    nc.vector.transpose(gt[0:32, b * 32 : (b + 1) * 32],
                                        s1col[b * 32 : (b + 1) * 32, 0:32])
            ps = ps_h.tile([P, NQ], fp32, tag="hb")
            for c, r in enumerate(range(4 * q, 4 * q + 4)):
                nc.tensor.matmul(
                    ps[:, c * P : (c + 1) * P],
                    lhsT=sel32[:, r * P : (r + 1) * P],
                    rhs=gt[:],
                    start=(c == 0),
                    stop=(c == 3),
                    skip_group_check=True,
                )
            with tc.high_priority() if q < 2 else nullcontext():
                nc.scalar.activation(
                    Gb[:, q * NQ : (q + 1) * NQ], ps[:], AF.Exp,
                    scale=1.0 - ALPHA,
                )

        # s1 quarters 0/1 with G chains; early s2 for the F scalars
        for j in range(4):
            emit_s1(j)
        for _ in range(4):
            warm_on(warm_rhs[:])
        emit_g_quarter(0)
        emit_s2(0, hi=True)
        emit_s2(1, hi=True)
        for j in range(4, 8):
            emit_s1(j)
        for _ in range(4):
            warm_on(warm_rhs[:])
        emit_g_quarter(1)
        for j in range(2, 6):
            emit_s2(j)

        # --- score tiles: quarters 0/1 narrow (early start), half-2 wide
        ptq = [[None] * NJ for _ in range(2)]
        ptH = [None] * NJ

        def emit_p(q, j):
            p = ppool.tile([P, NQ], bf16, tag="pq", name=f"p{q}_{j}")
            nc.vector.tensor_scalar(
                out=p[:],
                in0=Gb[:, q * NQ : (q + 1) * NQ],
                scalar1=F1col[:, j : j + 1],
                scalar2=F1acol[:, j : j + 1],
                op0=OP.mult,
                op1=OP.max,
            )
            ptq[q][j] = p

        def emit_pH(j):
            p = ppool.tile([P, 2 * NQ], bf16, tag="ph", name=f"ph_{j}")
            nc.vector.tensor_scalar(
                out=p[:],
                in0=Gb[:, 2 * NQ : 4 * NQ],
                scalar1=F1col[:, j : j + 1],
                scalar2=F1acol[:, j : j + 1],
                op0=OP.mult,
                op1=OP.max,
            )
            ptH[j] = p

        # quarter 0 tiles; remaining s2 dots hide behind
        for j in range(NJ):
            emit_p(0, j)
            if j + 6 < NJ:
                emit_s2(j + 6)
        for j in range(4, NJ):
            cast_x(j)
        # quarter 1 tiles; s1 half-2 dots + G half-2 chains hide here
        for j in range(NJ):
            emit_p(1, j)
            if j < 8:
                emit_s1(8 + j)
            if j == 9:
                emit_g_quarter(2)
                emit_g_quarter(3)
        for j in range(NJ):
            emit_pH(j)

        # --- PV + r + normalize, one PSUM group per i-quarter ------------
        out_eng = [nc.sync, nc.scalar]
        for g, (g0, g1_) in enumerate(GROUPS):
            outps = [
                ps_out.tile([P, C], fp32, tag="ops", name=f"ops_{g0}_{ki}")
                for ki in range(4)
            ]
            rps = ps_r.tile([P, C], fp32, tag="rps")
            for j in range(NJ):
                first, last = j == 0, j == NJ - 1
                for ki in range(4):
                    if g < 2:
                        lhs = ptq[g][j][:, ki * P : (ki + 1) * P]
                    else:
                        lhs = ptH[j][:, (g - 2) * NQ + ki * P : (g - 2) * NQ + (ki + 1) * P]
                    nc.tensor.matmul(
                        outps[ki][:], lhsT=lhs, rhs=xbf[j][:], start=first, stop=last
                    )
                    # start=True clears the WHOLE bank's has_written bits, so
                    # only the very first matmul into this bank may set it.
                    nc.tensor.matmul(
                        rps[:, ki : ki + 1],
                        lhsT=lhs,
                        rhs=ones_bf[:],
                        start=first and ki == 0,
                        stop=last,
                        skip_group_check=True,
                    )
            with tc.high_priority():
                nc.vector.reciprocal(rinv[:, g0:g1_], rps[:, 0:4])
            for ki, k in enumerate(range(g0, g1_)):
                ob = osb.tile([P, C], fp32, tag="ob")
                if g == 3 and ki % 2 == 1:
                    # last group: split normalizes ACT/DVE to shorten the tail
                    nc.vector.tensor_scalar_mul(
                        ob[:], outps[ki][:], rinv[:, k : k + 1]
                    )
                else:
                    nc.scalar.activation(
                        ob[:], outps[ki][:], AF.Copy, bias=0.0,
                        scale=rinv[:, k : k + 1],
                    )
                out_eng[ki % 2].dma_start(o_d[k * P : (k + 1) * P, :], ob[:])

    nc.compile()
    return nc


def _make_sel():
    # sel[k, j*128+m] = 1 if k == j else 0  (k<32, j<16)
    s = np.zeros((32, NJ * P), dtype=np.float16)
    for j in range(NJ):
        s[j, j * P : (j + 1) * P] = 1.0
    return s


def _get_nc():
    if "nc" not in _CACHE:
        _CACHE["nc"] = _build()
    return _CACHE["nc"]


def _run(inputs, trace=False, tmpdir=None):
    from concourse.bass_utils import run_bass_kernel_spmd

    nc = _get_nc()
    x = np.ascontiguousarray(np.asarray(inputs["x"], dtype=np.float32))
    w = np.ascontiguousarray(np.asarray(inputs["w"], dtype=np.float32))
    a = np.ascontiguousarray(np.asarray(inputs["a"], dtype=np.float32))
    sel = _make_sel()
    core_ids = list(range(B))
    in_maps = [
        {"x": x[b], "w": w, "a": a, "sel": sel, "sel32": sel.astype(np.float32)}
        for b in core_ids
    ]
    res = run_bass_kernel_spmd(nc, in_maps, core_ids, trace=trace, tmpdir=tmpdir)
    out = np.stack([res.results[b]["o"] for b in core_ids], axis=0)
    return out, res


def kernel(**inputs) -> np.ndarray:
    out, _ = _run(inputs, trace=False)
    return out
